# revision 19
# baseline (speedup 1.0000x reference)
"""Trainium2 kernel for nn_CachedReadoutModel (PCA -> MLP -> species shift -> segment sum).

Strategy (8 NeuronCores, data-parallel over atoms):
  host:  fold PCA into layer 1 (W_eff = (W1 @ pca_components).T, scaled x8 into
         fp8e4m3; the Act engine un-scales via scale=0.125); slice 1M atoms into
         8 ranges; STABLE-SORT each range by batch_map so every graph's atoms
         are contiguous; stage x transposed (feature-major) in fp8 as two
         planes (features 0..127 and 128..191); stage the per-atom species
         shift c[a] = shifts[argmax(node_attrs[a])] + b2 as an fp16 table.
  core:  per 1024-atom half: one DoubleRow fp8 matmul pair (K=256, features
         padded with zero weights) -> h; silu on Act (bias folded); 8 small
         matmuls with silu as stationary -> mlp in [atom, tile] layout;
         tot = mlp + c (DVE); per-superblock prefix sums along atoms via one
         triangular-matrix matmul; accumulate 128 tile-columns per PSUM bank,
         copy to SBUF, DMA out the per-tile inclusive prefix P[128, T] fp32.
  host:  per-graph sums from prefix differences at graph boundaries
         (tile bases accumulated in float64), sum cores, add base_energy.
"""

import os
import sys

for _p in ("/opt/trn_rl_repo", "/root/.axon_site/_ro/trn_rl_repo"):
    if os.path.isdir(_p) and _p not in sys.path:
        sys.path.insert(0, _p)

from contextlib import ExitStack

import numpy as np
import ml_dtypes

import concourse.bass as bass
import concourse.tile as tile
from concourse import bacc, mybir
from concourse._compat import with_exitstack
from concourse.bass_utils import run_bass_kernel_spmd

dt = mybir.dt
Alu = mybir.AluOpType
Act = mybir.ActivationFunctionType
PerfMode = mybir.MatmulPerfMode

N_ATOMS = 1_000_000
N_GRAPHS = 16384
N_CORES = 8
T = 992  # tiles of 128 atoms per core; A = 126976 >= ceil(1e6/8)
A = 128 * T
SB = T // 16  # superblocks of 2048 atoms

NP_F8 = mybir.dt.np(dt.float8e4)

_PROGRAM_CACHE = {}


@with_exitstack
def _emit_body(ctx: ExitStack, tc, ins, e_out):
    nc = tc.nc

    const = ctx.enter_context(tc.tile_pool(name="const", bufs=1))
    xpool = ctx.enter_context(tc.tile_pool(name="xpool", bufs=4))
    work = ctx.enter_context(tc.tile_pool(name="work", bufs=3))
    totp = ctx.enter_context(tc.tile_pool(name="totp", bufs=2))
    outp = ctx.enter_context(tc.tile_pool(name="outp", bufs=2))
    hps = ctx.enter_context(tc.tile_pool(name="hps", bufs=2, space="PSUM"))
    mlpps = ctx.enter_context(tc.tile_pool(name="mlpps", bufs=2, space="PSUM"))
    prefps = ctx.enter_context(tc.tile_pool(name="prefps", bufs=2, space="PSUM"))

    def load_const(name, shape, dtype):
        t = const.tile(shape, dtype, tag=name)
        nc.sync.dma_start(t[:], ins[name])
        return t

    wdr = load_const("wdr", [96, 256], dt.float8e4)
    w2c = load_const("w2c", [128, 1], dt.float8e4)
    w2c16 = load_const("w2c16", [128, 1], dt.float16)
    beff = load_const("beff", [128, 1], dt.float32)
    beff2 = load_const("beff2", [128, 1], dt.float32)
    tri = load_const("tri", [128, 128], dt.float16)
    ct = load_const("ct", [128, T], dt.float16)
    wdr3 = wdr[:].rearrange("p (j m) -> p j m", j=2)

    pref = None
    x3 = None
    for s in range(SB):
        mlp_ps = mlpps.tile([128, 16], dt.float32, tag="mlp")
        if s % 8 == 0:
            pref = prefps.tile([128, 128], dt.float32, tag="pref")
        if s % 4 == 0:
            # one x block = 4 superblocks (8192 atoms) -> 16KB DMA runs.
            # K=192 split as 2 interleaved subtiles of 96 partitions: feature
            # j*96+p lives at byte 2a+j of partition p. No pad rows.
            b0 = s * 2048
            nsb = min(4, SB - s)
            w_blk = nsb * 2048
            xt = xpool.tile([96, 2 * 8192], dt.float8e4, tag="x")
            x3 = xt[:].rearrange("p (n j) -> p j n", j=2)
            # split 2x2 (partition range x column half) so 4 DMA queues run
            # in parallel; one big DMA serializes on a single queue
            wh = w_blk  # interleaved bytes per half-column split
            for pr in (slice(0, 48), slice(48, 96)):
                for co in range(2):
                    nc.sync.dma_start(
                        xt[pr, co * wh : (co + 1) * wh],
                        ins["xdi"][pr, 2 * b0 + co * wh : 2 * b0 + (co + 1) * wh],
                    )
        for half in range(2):
            o0 = (s % 4) * 2048 + half * 1024
            h_ps = hps.tile([128, 1024], dt.float32, tag="h")
            for q in range(2):
                nc.tensor.matmul(
                    h_ps[:, q * 512 : (q + 1) * 512],
                    wdr3,
                    x3[:, :, o0 + q * 512 : o0 + (q + 1) * 512],
                    start=True,
                    stop=True,
                    perf_mode=PerfMode.DoubleRow,
                )
            # silu split: Act does 3 of 4 halves; DVE does every 4th as
            # hard-silu zf*clamp(zf/4+0.5,0,1) via t = z/32 + (beff/4+0.5)
            on_dve = (s * 2 + half) % 6 == 5
            if on_dve:
                t16 = work.tile([128, 1024], dt.float16, tag="t16")
                nc.vector.tensor_scalar(t16[:], h_ps[:], 1.0 / 32.0, beff2[:], Alu.mult, Alu.add)
                u16 = work.tile([128, 1024], dt.float16, tag="u16")
                nc.vector.tensor_scalar(u16[:], t16[:], 0.0, 1.0, Alu.max, Alu.min)
                v16 = work.tile([128, 1024], dt.float16, tag="v16")
                nc.vector.tensor_scalar(v16[:], t16[:], 4.0, -2.0, Alu.mult, Alu.add)
                silu = work.tile([128, 1024], dt.float16, tag="silu16")
                nc.vector.tensor_tensor(silu[:], u16[:], v16[:], Alu.mult)
                w2 = w2c16
            else:
                silu = work.tile([128, 1024], dt.float8e4, tag="silu")
                nc.scalar.activation(silu[:], h_ps[:], Act.Silu, bias=beff[:], scale=0.125)
                w2 = w2c
            for j in range(8):
                k = half * 8 + j
                nc.tensor.matmul(
                    mlp_ps[:, k : k + 1],
                    silu[:, j * 128 : (j + 1) * 128],
                    w2[:],
                    start=True,
                    stop=True,
                )
        # mlp is x64 (w2c staged x64); ct is staged x64; tri entries are 1/64
        tot = totp.tile([128, 16], dt.float16, tag="tot")
        nc.vector.tensor_tensor(tot[:], mlp_ps[:], ct[:, s * 16 : (s + 1) * 16], Alu.add)
        c0 = (s % 8) * 16
        nc.tensor.matmul(pref[:, c0 : c0 + 16], tri[:], tot[:], start=True, stop=True)
        if s % 8 == 7 or s == SB - 1:
            w = c0 + 16
            b = s // 8
            ob = outp.tile([128, 128], dt.float32, tag="ob")
            nc.vector.tensor_copy(ob[:, 0:w], pref[:, 0:w])
            nc.sync.dma_start(e_out[:, b * 128 : b * 128 + w], ob[:, 0:w])


def _build_program():
    nc = bacc.Bacc("TRN2", target_bir_lowering=False, debug=False)
    shapes = {
        "xdi": ([96, 2 * A], dt.float8e4),
        "wdr": ([96, 256], dt.float8e4),
        "w2c": ([128, 1], dt.float8e4),
        "w2c16": ([128, 1], dt.float16),
        "beff": ([128, 1], dt.float32),
        "beff2": ([128, 1], dt.float32),
        "tri": ([128, 128], dt.float16),
        "ct": ([128, T], dt.float16),
    }
    ins = {name: nc.declare_dram_parameter(name, list(sh), d, isOutput=False).ap() for name, (sh, d) in shapes.items()}
    e_out = nc.declare_dram_parameter("e_out", [128, T], dt.float32, isOutput=True).ap()
    with tile.TileContext(nc) as tc:
        _emit_body(tc, ins, e_out)
    nc.finalize()
    return nc


def _stage_params(pca_mean, pca_components, W1, b1, W2, b2, shifts):
    W_eff = (W1.astype(np.float64) @ pca_components.astype(np.float64)).T  # [192, 128]
    b_eff = b1.astype(np.float64) - W_eff.T @ pca_mean.astype(np.float64)
    W8 = (W_eff * 8.0).astype(np.float32).astype(NP_F8)  # Act un-scales via scale=0.125
    wdr = np.zeros((96, 256), dtype=NP_F8)
    wdr[:, 0:128] = W8[0:96]
    wdr[:, 128:256] = W8[96:192]
    return {
        "wdr": wdr,
        "w2c": np.ascontiguousarray(W2.reshape(128, 1) * 64.0).astype(NP_F8),
        "w2c16": np.ascontiguousarray(W2.reshape(128, 1) * 64.0).astype(np.float16),
        "beff": b_eff.astype(np.float32).reshape(128, 1),
        "beff2": (b_eff * 0.25 + 0.5).astype(np.float32).reshape(128, 1),
        "tri": np.triu(np.full((128, 128), 1.0 / 64.0, dtype=np.float16)),
    }


def _stage_core_inputs(x_c, c_vals, bm_c):
    """Sort one core's atoms by graph, pad to A, build device arrays."""
    n = x_c.shape[0]
    perm = np.argsort(bm_c, kind="stable")
    bm_s = bm_c[perm]

    xt = np.zeros((192, A), dtype=NP_F8)
    xt[:, :n] = x_c[perm].T.astype(NP_F8)
    xdi = np.empty((96, 2 * A), dtype=NP_F8)
    xdi[:, 0::2] = xt[0:96]
    xdi[:, 1::2] = xt[96:192]
    cpad = np.zeros(A, dtype=np.float16)
    cpad[:n] = c_vals[perm]
    return (
        {
            "xdi": xdi,
            "ct": np.ascontiguousarray(cpad.reshape(T, 128).T),
        },
        bm_s,
    )


def _get_program():
    if T not in _PROGRAM_CACHE:
        _PROGRAM_CACHE[T] = _build_program()
    return _PROGRAM_CACHE[T]


def kernel(x, node_attrs, batch_map, base_energy, pca_mean, pca_components, W1, b1, W2, b2, shifts, _trace=False):
    x = np.asarray(x, dtype=np.float32)
    node_attrs = np.asarray(node_attrs, dtype=np.float32)
    batch_map = np.asarray(batch_map).astype(np.int64)
    base_energy = np.asarray(base_energy, dtype=np.float32)
    shifts = np.asarray(shifts, np.float32)
    b2 = np.asarray(b2, np.float32)
    params = _stage_params(
        np.asarray(pca_mean, np.float32),
        np.asarray(pca_components, np.float32),
        np.asarray(W1, np.float32),
        np.asarray(b1, np.float32),
        np.asarray(W2, np.float32),
        b2,
        shifts,
    )
    c_all = ((shifts[np.argmax(node_attrs, axis=1)] + b2[0]) * 64.0).astype(np.float16)

    n = x.shape[0]
    bounds = [min((n + N_CORES - 1) // N_CORES * c, n) for c in range(N_CORES + 1)]
    in_maps, bms = [], []
    for c in range(N_CORES):
        s, e = bounds[c], bounds[c + 1]
        m, bm_s = _stage_core_inputs(x[s:e], c_all[s:e], batch_map[s:e])
        m.update(params)
        in_maps.append(m)
        bms.append(bm_s)

    nc = _get_program()
    res = run_bass_kernel_spmd(nc, in_maps, list(range(N_CORES)), trace=_trace)
    delta = np.zeros(N_GRAPHS, dtype=np.float64)
    for c in range(N_CORES):
        bm_s = bms[c]
        nn = len(bm_s)
        if nn == 0:
            continue
        P = np.asarray(res.results[c]["e_out"], dtype=np.float64)  # [128, T]
        tile_base = np.concatenate(([0.0], np.cumsum(P[127, :])))
        ends_mask = np.empty(nn, dtype=bool)
        ends_mask[:-1] = bm_s[1:] != bm_s[:-1]
        ends_mask[-1] = True
        ends = np.flatnonzero(ends_mask)
        Gv = tile_base[ends // 128] + P[ends % 128, ends // 128]
        deltas = np.diff(np.concatenate(([0.0], Gv)))
        delta[bm_s[ends]] += deltas
    delta = delta.astype(np.float32)
    final = base_energy + delta
    if _trace:
        kernel._last_result = res
    return final, delta
